# revision 1
# baseline (speedup 1.0000x reference)
"""GNN message-passing (graph convolution) kernel for 8 Trainium2 NeuronCores.

    out = relu(segment_sum(h[col], row) + bias),  h = x @ W

Strategy (dst-block sharding — no collectives needed):
  * Host sorts edges by destination node and buckets them into 157 blocks of
    128 dst nodes; blocks are assigned contiguously to cores (20/core).  Each
    core produces a disjoint slice of the output, so partial aggregates never
    need an all-reduce.
  * Phase A (per core, replicated): h = x @ W on the PE in fp16
    (PSUM fp32 accumulate), streamed to a per-core DRAM buffer h[20096,128]
    fp16.  x is shipped pre-transposed/pre-tiled from the host so each lhsT
    tile is one contiguous 64KB DMA.
  * Phase B: for each dst block, dma_gather (SWDGE) fetches the h rows of the
    block's (padded) edge list into SBUF with edge-on-partition layout
    [128e, PB, 128f]; the DVE builds one-hot tiles S[e,n] = (iota == rowloc)
    in fp16; the PE computes out_block += S^T @ val accumulating all chunks of
    the block in PSUM fp32 — an exact segment-sum.  Bias is folded in as an
    extra "bias chunk" per block (gathers a bias row stored at h[20095] with an
    identity one-hot).  ACT applies ReLU PSUM->SBUF, then the result is DMA'd
    out.

Numerics: fp16 operands with fp32 accumulation everywhere; one-hot matmul is
exact, so the only error is fp16 rounding of x, W and h (~1e-3 relative).
"""

import sys

import numpy as np

sys.path.insert(0, "/opt/trn_rl_repo")

import concourse.bacc as bacc  # noqa: E402
import concourse.bass as bass  # noqa: E402  (engine types)
import concourse.mybir as mybir  # noqa: E402
from concourse.bass_utils import run_bass_kernel_spmd  # noqa: E402

N_NODES = 20000
FIN = 256
FOUT = 128
N_EDGES = 640000

NT = 157                 # node tiles of 128 (nodes padded to 20096)
NPAD = NT * 128          # 20096
NBLK = 157               # dst blocks of 128 nodes
NCORES = 8
NB = 20                  # block slots per core (core 7: 17 real + 3 dummy)
BIAS_ROW = NPAD - 1      # h row that phase-B reads the bias vector from

XT_BUFS = 4              # xT tile ring (phase A)
H_BUFS = 4               # h sbuf tile ring (phase A)
S_BUFS = 4               # one-hot tile ring (phase B)

FP16 = mybir.dt.float16
FP32 = mybir.dt.float32
I16 = mybir.dt.int16


def _host_prep(x, edge_index, weight, bias):
    """Cast/retile operands and bucket edges by destination block."""
    x = np.asarray(x, np.float32)
    weight = np.asarray(weight, np.float32)
    bias = np.asarray(bias, np.float32)

    xpad = np.zeros((NPAD, FIN), np.float32)
    xpad[:N_NODES] = x
    # lhsT tiles: xt_tiles[i, k, kc, n] = x[i*128+n, kc*128+k]
    xt_tiles = np.ascontiguousarray(
        xpad.reshape(NT, 128, 2, 128).transpose(0, 3, 2, 1).astype(np.float16)
    )
    w_t = np.ascontiguousarray(weight.astype(np.float16).reshape(2, 128, 128))
    bias16 = np.ascontiguousarray(bias.astype(np.float16).reshape(1, 128))
    iota16 = np.ascontiguousarray(
        np.broadcast_to(np.arange(128, dtype=np.float16), (128, 128))
    )

    row = np.asarray(edge_index[0]).astype(np.int64)
    col = np.asarray(edge_index[1]).astype(np.int64)
    order = np.argsort(row, kind="stable")
    rs = row[order].astype(np.int32)
    cs = col[order].astype(np.int32)

    blk = rs >> 7
    counts = np.bincount(blk, minlength=NBLK)
    starts = np.concatenate([[0], np.cumsum(counts)])
    pb = int(np.max((counts + 127) // 128)) + 1  # +1 for the bias chunk
    pb = ((pb + 6) // 7) * 7  # sub-gathers of 7 chunks (896 idxs <= SWDGE ring)
    nidx = pb * 128
    idxc = nidx // 16

    col16 = np.zeros((NCORES, 128, NB * idxc), np.int16)
    rloc16 = np.full((NCORES, 128, NB * pb), -1.0, np.float32)
    bias_rl = np.arange(128, dtype=np.float32)
    for c in range(NCORES):
        for s in range(NB):
            b = c * NB + s
            lin_col = np.zeros(nidx, np.int32)
            lin_rl = np.full(nidx, -1.0, np.float32)
            lin_col[:128] = BIAS_ROW          # bias chunk: identity one-hot
            lin_rl[:128] = bias_rl
            if b < NBLK:
                e0, e1 = int(starts[b]), int(starts[b + 1])
                k = e1 - e0
                lin_col[128:128 + k] = cs[e0:e1]
                lin_rl[128:128 + k] = rs[e0:e1] - b * 128
            # the SWDGE tx/rx Q7 pair read the indices from different
            # 16-partition groups — replicate the 16-row wrap to all 128
            col16[c, :, s * idxc:(s + 1) * idxc] = np.tile(
                lin_col.reshape(idxc, 16).T.astype(np.int16), (8, 1)
            )
            rloc16[c, :, s * pb:(s + 1) * pb] = (
                lin_rl.reshape(pb, 128).T.astype(np.float32)
            )
    return xt_tiles, w_t, bias16, iota16, col16, rloc16, pb


def _build_program(pb):
    nidx = pb * 128
    idxc = nidx // 16
    nc = bacc.Bacc("TRN2")

    xt_d = nc.dram_tensor("xt", [NT, 128, 2, 128], FP16, kind="ExternalInput")
    w_d = nc.dram_tensor("w", [2, 128, 128], FP16, kind="ExternalInput")
    b_d = nc.dram_tensor("bias", [1, 128], FP16, kind="ExternalInput")
    io_d = nc.dram_tensor("iota", [128, 128], FP16, kind="ExternalInput")
    col_d = nc.dram_tensor("col", [128, NB * idxc], I16, kind="ExternalInput")
    rl_d = nc.dram_tensor("rl", [128, NB * pb], FP32, kind="ExternalInput")
    h_d = nc.dram_tensor("hbuf", [NPAD, 128], FP16)
    o_d = nc.dram_tensor("out", [NB * 128, 128], FP32, kind="ExternalOutput")

    from contextlib import ExitStack

    with ExitStack() as es:
        ph0 = es.enter_context(nc.psum_tensor("ph0", [128, 512], FP32))
        ph1 = es.enter_context(nc.psum_tensor("ph1", [128, 512], FP32))
        ph2 = es.enter_context(nc.psum_tensor("ph2", [128, 512], FP32))
        ph3 = es.enter_context(nc.psum_tensor("ph3", [128, 512], FP32))
        pb0 = es.enter_context(nc.psum_tensor("pb0", [128, 512], FP32))
        pb1 = es.enter_context(nc.psum_tensor("pb1", [128, 512], FP32))
        w_sb = es.enter_context(nc.sbuf_tensor("w_sb", [128, 2, 128], FP16))
        xt_sb = es.enter_context(
            nc.sbuf_tensor("xt_sb", [128, XT_BUFS, 2, 128], FP16)
        )
        h_sb = es.enter_context(nc.sbuf_tensor("h_sb", [128, H_BUFS, 128], FP16))
        iota_sb = es.enter_context(nc.sbuf_tensor("iota_sb", [128, 128], FP16))
        col_sb = es.enter_context(nc.sbuf_tensor("col_sb", [128, NB * idxc], I16))
        rl_sb = es.enter_context(nc.sbuf_tensor("rl_sb", [128, NB * pb], FP32))
        val_sb = es.enter_context(
            nc.sbuf_tensor("val_sb", [128, 2, pb, 128], FP16)
        )
        s_sb = es.enter_context(nc.sbuf_tensor("s_sb", [128, S_BUFS, 128], FP16))
        o_sb = es.enter_context(nc.sbuf_tensor("o_sb", [128, 2, 128], FP32))
        # DMA-completion sems rotate per ring slot (DMA completions on one
        # sem can reorder, so each slot gets its own counter).
        s_ld = [es.enter_context(nc.semaphore(f"s_ld{k}")) for k in range(5)]
        s_xt = [es.enter_context(nc.semaphore(f"s_xt{k}")) for k in range(XT_BUFS)]
        s_hw = [es.enter_context(nc.semaphore(f"s_hw{k}")) for k in range(H_BUFS)]
        s_bw = es.enter_context(nc.semaphore("s_bw"))
        s_gat = [
            es.enter_context(nc.semaphore(f"s_gat{k}"))
            for k in range(2 * (pb // 7))
        ]
        s_ow = [es.enter_context(nc.semaphore(f"s_ow{k}")) for k in range(2)]
        # compute-engine sems increment in program order (no ambiguity)
        s_hmm = es.enter_context(nc.semaphore("s_hmm"))
        s_hcp = es.enter_context(nc.semaphore("s_hcp"))
        s_s = es.enter_context(nc.semaphore("s_s"))
        s_pmm = es.enter_context(nc.semaphore("s_pmm"))
        s_ocp = es.enter_context(nc.semaphore("s_ocp"))
        block = es.enter_context(nc.Block())
        ph = [ph0, ph1, ph2, ph3]
        pbk = [pb0, pb1]

        hw_total = [16 * len(range(k, NT, H_BUFS)) for k in range(H_BUFS)]

        def store_h(sync, j):
            sync.wait_ge(s_hcp, j + 1)
            sync.dma_start(
                h_d[j * 128:(j + 1) * 128, :], h_sb[:, j % H_BUFS, :]
            ).then_inc(s_hw[j % H_BUFS], 16)

        @block.sync
        def _(sync):
            # one-time loads
            sync.dma_start(w_sb[:, 0, :], w_d[0]).then_inc(s_ld[0], 16)
            sync.dma_start(w_sb[:, 1, :], w_d[1]).then_inc(s_ld[1], 16)
            sync.dma_start(iota_sb[:, :], io_d[:, :]).then_inc(s_ld[2], 16)
            sync.dma_start(col_sb[:, :], col_d[:, :]).then_inc(s_ld[3], 16)
            sync.dma_start(rl_sb[:, :], rl_d[:, :]).then_inc(s_ld[4], 16)
            # phase A: stream xT tiles in, h tiles out (staggered)
            for i in range(NT):
                if i >= XT_BUFS:
                    sync.wait_ge(s_hmm, i - (XT_BUFS - 1))
                sync.dma_start(xt_sb[:, i % XT_BUFS, :, :], xt_d[i]).then_inc(
                    s_xt[i % XT_BUFS], 16
                )
                if i >= 3:
                    store_h(sync, i - 3)
            for j in range(NT - 3, NT):
                store_h(sync, j)
            # bias row (after ALL h writes are complete — tile 156 covers it)
            for k in range(H_BUFS):
                sync.wait_ge(s_hw[k], hw_total[k])
            sync.dma_start(h_d[BIAS_ROW:BIAS_ROW + 1, :], b_d[0:1, :]).then_inc(
                s_bw, 16
            )
            # phase B: output stores
            for b in range(NB):
                sync.wait_ge(s_ocp, b + 1)
                sync.dma_start(
                    o_d[b * 128:(b + 1) * 128, :], o_sb[:, b % 2, :]
                ).then_inc(s_ow[b % 2], 16)

        @block.gpsimd
        def _(gpsimd):
            gpsimd.wait_ge(s_ld[3], 16)
            for k in range(H_BUFS):
                gpsimd.wait_ge(s_hw[k], hw_total[k])
            gpsimd.wait_ge(s_bw, 16)
            for b in range(NB):
                if b >= 2:
                    gpsimd.wait_ge(s_pmm, (b - 1) * pb)
                for g in range(pb // 7):
                    gpsimd.dma_gather(
                        val_sb[:, b % 2, g * 7:(g + 1) * 7, :],
                        h_d[:, :],
                        col_sb[:, b * idxc + g * 56:b * idxc + (g + 1) * 56],
                        896,
                        896,
                        128,
                    ).then_inc(s_gat[(b % 2) * (pb // 7) + g], 16)

        @block.tensor
        def _(tensor):
            for k in range(2):
                tensor.wait_ge(s_ld[k], 16)
            # phase A: h tile i = xT_i^T @ W  (two K chunks)
            for i in range(NT):
                tensor.wait_ge(s_xt[i % XT_BUFS], 16 * (i // XT_BUFS + 1))
                if i >= XT_BUFS:
                    tensor.wait_ge(s_hcp, i - (XT_BUFS - 1))
                tensor.matmul(
                    ph[i % XT_BUFS][:, 0:128],
                    xt_sb[:, i % XT_BUFS, 0, :],
                    w_sb[:, 0, :],
                    start=True,
                    stop=False,
                )
                tensor.matmul(
                    ph[i % XT_BUFS][:, 0:128],
                    xt_sb[:, i % XT_BUFS, 1, :],
                    w_sb[:, 1, :],
                    start=False,
                    stop=True,
                ).then_inc(s_hmm, 1)
            # phase B: out_block += S_chunk^T @ val_chunk
            for b in range(NB):
                if b >= 2:
                    tensor.wait_ge(s_ocp, b - 1)
                for c in range(pb):
                    j = b * pb + c
                    if c % 7 == 0:
                        tensor.wait_ge(
                            s_gat[(b % 2) * (pb // 7) + c // 7],
                            16 * (b // 2 + 1),
                        )
                    tensor.wait_ge(s_s, j + 1)
                    tensor.matmul(
                        pbk[b % 2][:, 0:128],
                        s_sb[:, j % S_BUFS, :],
                        val_sb[:, b % 2, c, :],
                        start=(c == 0),
                        stop=(c == pb - 1),
                    ).then_inc(s_pmm, 1)

        @block.vector
        def _(vector):
            # phase A: PSUM fp32 -> SBUF fp16
            for i in range(NT):
                vector.wait_ge(s_hmm, i + 1)
                if i >= H_BUFS:
                    vector.wait_ge(s_hw[i % H_BUFS], 16 * (i // H_BUFS))
                vector.tensor_copy(
                    h_sb[:, i % H_BUFS, :], ph[i % XT_BUFS][:, 0:128]
                ).then_inc(s_hcp, 1)
            # phase B: one-hot tiles S[e, n] = (iota[n] == rowloc[e])
            vector.wait_ge(s_ld[2], 16)
            vector.wait_ge(s_ld[4], 16)
            for j in range(NB * pb):
                if j >= S_BUFS:
                    vector.wait_ge(s_pmm, j - (S_BUFS - 1))
                vector.tensor_scalar(
                    s_sb[:, j % S_BUFS, :],
                    iota_sb[:, :],
                    rl_sb[:, j:j + 1],
                    None,
                    mybir.AluOpType.is_equal,
                ).then_inc(s_s, 1)

        @block.scalar
        def _(scalar):
            for b in range(NB):
                scalar.wait_ge(s_pmm, (b + 1) * pb)
                if b >= 2:
                    scalar.wait_ge(s_ow[b % 2], 16 * (b // 2))
                scalar.activation(
                    o_sb[:, b % 2, :],
                    pbk[b % 2][:, 0:128],
                    mybir.ActivationFunctionType.Relu,
                ).then_inc(s_ocp, 1)

    nc.compile()
    return nc


def _run(x, edge_index, weight, bias, trace=False):
    xt_tiles, w_t, bias16, iota16, col16, rloc16, pb = _host_prep(
        x, edge_index, weight, bias
    )
    nc = _build_program(pb)
    in_maps = [
        {
            "xt": xt_tiles,
            "w": w_t,
            "bias": bias16,
            "iota": iota16,
            "col": np.ascontiguousarray(col16[c]),
            "rl": np.ascontiguousarray(rloc16[c]),
        }
        for c in range(NCORES)
    ]
    res = run_bass_kernel_spmd(nc, in_maps, list(range(NCORES)), trace=trace)
    out = np.concatenate([res.results[c]["out"] for c in range(NCORES)], axis=0)
    return np.ascontiguousarray(out[:N_NODES]), res


def kernel(x, edge_index, weight, bias):
    out, _ = _run(x, edge_index, weight, bias, trace=False)
    return out



# revision 14
# speedup vs baseline: 1.8799x; 1.8799x over previous
"""GNN message-passing (graph convolution) kernel for 8 Trainium2 NeuronCores.

    out = relu(segment_sum(h[col], row) + bias),  h = x @ W

Strategy (dst-block sharding -- no collectives):
  * Host sorts edges by destination node into 157 blocks of 128 dst nodes;
    blocks are assigned contiguously to cores (20/core).  Within a core the
    blocks are ordered biggest-first into "slots" so that the per-slot
    num_idxs (max over cores) carries minimal padding; the host un-permutes
    the output rows afterwards.
  * Phase A (replicated): h = x @ W on the PE in fp16 (PSUM fp32).  x ships
    pre-transposed partition-major [128k, NT, 2, 128n] so slab loads are
    16-tile (8KB-elem) DMAs.  h is written to DRAM TILE-MAJOR (row p*NT+t
    holds node t*128+p) so slab stores are 4KB-contiguous per partition --
    twice the DMA rate of row-major 256B stores.  Gather indices are
    permuted on the host to match (idx' = (r%128)*NT + r//128).
  * Phase B: one SWDGE dma_gather per dst-block slot (~4.3K idxs) fetches
    h rows edge-on-partition [128e, chunks, 128f]; the DVE builds one-hot
    tiles S[e,n] = (iota == rowloc) fp16; the PE computes
    out_block += S^T @ val accumulated over chunks in PSUM fp32 (an exact
    segment-sum).  Bias rides as 128 leading idxs per slot that gather a
    bias row stored at the end of h with an identity one-hot.  ACT applies
    ReLU PSUM->SBUF; results DMA out per slot.

Numerics: fp16 operands with fp32 accumulation everywhere; one-hot matmul is
exact, so the only error is fp16 rounding of x, W and h (~3e-4 relative).
"""

import sys

import numpy as np

sys.path.insert(0, "/opt/trn_rl_repo")

import concourse.bacc as bacc  # noqa: E402
import concourse.mybir as mybir  # noqa: E402
from concourse.bass_utils import run_bass_kernel_spmd  # noqa: E402

N_NODES = 20000
FIN = 256
FOUT = 128
N_EDGES = 640000

NT = 157                 # node tiles of 128 (nodes padded to 20096)
NPAD = NT * 128          # 20096
NBLK = 157               # dst blocks of 128 nodes
NCORES = 8
NB = 20                  # block slots per core (core 7: 17 real + 3 dummy)
BIAS_NODE = NPAD - 1     # node whose h row phase-B reads the bias vector from

SLAB = 16                # phase-A tiles per DMA slab
PSG = 4                  # phase-A tiles per PSUM bank group (cast batch)
XT_BUFS = 4              # xt slab ring (phase A)
PH_BANKS = 4             # phase-A PSUM banks
VAL_BUFS = 3             # gathered-value ring (phase B)
S_BUFS = 32              # one-hot tile ring (phase B)

FP16 = mybir.dt.float16
FP32 = mybir.dt.float32
I16 = mybir.dt.int16


def _idxp(r):
    """Node index -> tile-major h row index: row = (r%128)*NT + r//128."""
    return (r % 128) * NT + r // 128


def _host_prep(x, edge_index, weight, bias):
    """Cast/retile operands and bucket edges by destination block."""
    x = np.asarray(x, np.float32)
    weight = np.asarray(weight, np.float32)
    bias = np.asarray(bias, np.float32)

    xpad = np.zeros((NPAD, FIN), np.float32)
    xpad[:N_NODES] = x
    # lhsT partition-major: xt[k, t, kc, n] = x[t*128+n, kc*128+k]
    xt_pm = np.ascontiguousarray(
        xpad.reshape(NT, 128, 2, 128).transpose(3, 0, 2, 1).astype(np.float16)
    )
    # W partition-major: w[k, kc, f] = weight[kc*128+k, f]
    w_pm = np.ascontiguousarray(
        weight.reshape(2, 128, 128).transpose(1, 0, 2).astype(np.float16)
    )
    bias16 = np.ascontiguousarray(bias.astype(np.float16).reshape(1, 128))
    iota16 = np.ascontiguousarray(
        np.broadcast_to(np.arange(128, dtype=np.float16), (128, 128))
    )

    row = np.asarray(edge_index[0]).astype(np.int64)
    col = np.asarray(edge_index[1]).astype(np.int64)
    order = np.argsort(row, kind="stable")
    rs = row[order].astype(np.int32)
    cs = col[order].astype(np.int32)

    blk = rs >> 7
    counts = np.bincount(blk, minlength=NBLK)
    starts = np.concatenate([[0], np.cumsum(counts)])

    # per-core slot assignment: biggest block first
    slot_blocks = np.full((NCORES, NB), -1, np.int64)  # block id per (c, s)
    for c in range(NCORES):
        blks = [b for b in range(c * NB, min((c + 1) * NB, NBLK))]
        blks.sort(key=lambda b: -counts[b])
        for s, b in enumerate(blks):
            slot_blocks[c, s] = b

    # A bias chunk per slot (128 gathered bias rows + identity one-hot) is
    # only needed when bias is nonzero; the harness's bias is zeros.
    use_bias = bool(np.any(bias != 0.0))
    boff = 128 if use_bias else 0

    # per-slot padded idx count: bias idxs + max block size, to mult of 128
    sizes = np.zeros((NCORES, NB), np.int64)
    for c in range(NCORES):
        for s in range(NB):
            b = slot_blocks[c, s]
            sizes[c, s] = counts[b] if b >= 0 else 0
    n128 = [
        max(128, int(-(-(boff + int(sizes[:, s].max())) // 128) * 128))
        for s in range(NB)
    ]
    chunks = [n // 128 for n in n128]

    tot_idx = sum(n128)
    tot_ch = sum(chunks)
    col16 = np.zeros((NCORES, 128, tot_idx // 16), np.int16)
    rl32 = np.full((NCORES, 128, tot_ch), -1.0, np.float32)
    bias_rl = np.arange(128, dtype=np.float32)
    pad_idx = _idxp(BIAS_NODE) if use_bias else 0

    ioff = 0
    choff = 0
    for s in range(NB):
        n = n128[s]
        ch = chunks[s]
        for c in range(NCORES):
            b = slot_blocks[c, s]
            lin_col = np.full(n, pad_idx, np.int32)
            lin_rl = np.full(ch * 128, -1.0, np.float32)
            if use_bias:
                lin_rl[:128] = bias_rl
            if b >= 0:
                e0, e1 = int(starts[b]), int(starts[b + 1])
                k = e1 - e0
                cols = cs[e0:e1]
                lin_col[boff:boff + k] = (cols % 128) * NT + cols // 128
                lin_rl[boff:boff + k] = rs[e0:e1] - b * 128
            # SWDGE reads idxs wrapped in 16 partitions, replicated to 128
            col16[c, :, ioff // 16:(ioff + n) // 16] = np.tile(
                lin_col.reshape(n // 16, 16).T.astype(np.int16), (8, 1)
            )
            rl32[c, :, choff:choff + ch] = lin_rl.reshape(ch, 128).T
        ioff += n
        choff += ch

    plan = {"n128": n128, "chunks": chunks, "slot_blocks": slot_blocks}
    return xt_pm, w_pm, bias16, iota16, col16, rl32, plan


def _build_program(plan):
    n128 = plan["n128"]
    chunks = plan["chunks"]
    chmax = max(chunks)
    tot_idx = sum(n128)
    tot_ch = sum(chunks)
    cum = np.concatenate([[0], np.cumsum(chunks)])  # cum[s] = chunks before s

    nc = bacc.Bacc("TRN2")

    xt_d = nc.dram_tensor("xt", [128, NT, 2, 128], FP16, kind="ExternalInput")
    w_d = nc.dram_tensor("w", [128, 2, 128], FP16, kind="ExternalInput")
    b_d = nc.dram_tensor("bias", [1, 128], FP16, kind="ExternalInput")
    io_d = nc.dram_tensor("iota", [128, 128], FP16, kind="ExternalInput")
    col_d = nc.dram_tensor("col", [128, tot_idx // 16], I16, kind="ExternalInput")
    rl_d = nc.dram_tensor("rl", [128, tot_ch], FP32, kind="ExternalInput")
    h_d = nc.dram_tensor("hbuf", [NPAD, 128], FP16)  # tile-major rows
    o_d = nc.dram_tensor("out", [NB, 128, 128], FP32, kind="ExternalOutput")

    # phase-A DMA slabs of up to SLAB tiles
    slabs = []
    t = 0
    while t < NT:
        nt = min(SLAB, NT - t)
        slabs.append((t, nt))
        t += nt
    nslab = len(slabs)
    # phase-A psum groups of up to PSG tiles (aligned inside slabs)
    pgroups = []
    for t0, nt in slabs:
        u = 0
        while u < nt:
            np_ = min(PSG, nt - u)
            pgroups.append((t0 + u, np_))
            u += np_
    npg = len(pgroups)
    # cast count per slab prefix (casts are 1 per pgroup)
    pg_of_tile_end = {}  # tile_end -> number of pgroups fully before it
    acc = 0
    for g0, gn in pgroups:
        acc += 1
        pg_of_tile_end[g0 + gn] = acc

    from contextlib import ExitStack

    with ExitStack() as es:
        ph = [
            es.enter_context(nc.psum_tensor(f"ph{k}", [128, 512], FP32))
            for k in range(PH_BANKS)
        ]
        pbk = [
            es.enter_context(nc.psum_tensor(f"pb{k}", [128, 512], FP32))
            for k in range(2)
        ]
        w_sb = es.enter_context(nc.sbuf_tensor("w_sb", [128, 2, 128], FP16))
        xt_sb = es.enter_context(
            nc.sbuf_tensor("xt_sb", [128, XT_BUFS, SLAB, 2, 128], FP16)
        )
        h16_sb = es.enter_context(
            nc.sbuf_tensor("h16_sb", [128, 2, SLAB, 128], FP16)
        )
        iota_sb = es.enter_context(nc.sbuf_tensor("iota_sb", [128, 128], FP16))
        col_sb = es.enter_context(
            nc.sbuf_tensor("col_sb", [128, tot_idx // 16], I16)
        )
        rl_sb = es.enter_context(nc.sbuf_tensor("rl_sb", [128, tot_ch], FP32))
        val_sb = es.enter_context(
            nc.sbuf_tensor("val_sb", [128, VAL_BUFS, chmax, 128], FP16)
        )
        s_sb = es.enter_context(nc.sbuf_tensor("s_sb", [128, S_BUFS, 128], FP16))
        o_sb = es.enter_context(nc.sbuf_tensor("o_sb", [128, 2, 128], FP32))

        s_ld = [es.enter_context(nc.semaphore(f"s_ld{k}")) for k in range(4)]
        s_xt = [
            es.enter_context(nc.semaphore(f"s_xt{k}")) for k in range(XT_BUFS)
        ]
        s_hw = [es.enter_context(nc.semaphore(f"s_hw{k}")) for k in range(2)]
        s_bw = es.enter_context(nc.semaphore("s_bw"))
        s_gat = [
            es.enter_context(nc.semaphore(f"s_gat{k}")) for k in range(VAL_BUFS)
        ]
        s_ow = [es.enter_context(nc.semaphore(f"s_ow{k}")) for k in range(2)]
        s_hmm = es.enter_context(nc.semaphore("s_hmm"))
        s_hcp = es.enter_context(nc.semaphore("s_hcp"))
        s_s = es.enter_context(nc.semaphore("s_s"))
        s_pmm = es.enter_context(nc.semaphore("s_pmm"))
        s_ocp = es.enter_context(nc.semaphore("s_ocp"))
        block = es.enter_context(nc.Block())

        h_tm = h_d[:, :].rearrange("(p t) f -> p t f", p=128)  # tile-major view
        hw_total = [0, 0]
        for k, (t0, nt) in enumerate(slabs):
            hw_total[k % 2] += 16

        @block.sync
        def _(sync):
            # one-time loads (xt slab 0 ahead of the big col table so the PE
            # starts early; col/rl only matter at the phase-B boundary)
            sync.dma_start(w_sb[:, :, :], w_d[:, :, :]).then_inc(s_ld[0], 16)
            sync.dma_start(iota_sb[:, :], io_d[:, :]).then_inc(s_ld[1], 16)

            def xt_load(i):
                t0, nt = slabs[i]
                if i >= XT_BUFS:
                    pt0, pnt = slabs[i - XT_BUFS]
                    sync.wait_ge(s_hmm, pt0 + pnt)
                sync.dma_start(
                    xt_sb[:, i % XT_BUFS, 0:nt, :, :], xt_d[:, t0:t0 + nt, :, :]
                ).then_inc(s_xt[i % XT_BUFS], 16)

            def h_store(k):
                kt0, knt = slabs[k]
                sync.wait_ge(s_hcp, pg_of_tile_end[kt0 + knt])
                sync.dma_start(
                    h_tm[:, kt0:kt0 + knt, :], h16_sb[:, k % 2, 0:knt, :]
                ).then_inc(s_hw[k % 2], 16)

            xt_load(0)
            xt_load(1)
            sync.dma_start(col_sb[:, :], col_d[:, :]).then_inc(s_ld[2], 16)
            sync.dma_start(rl_sb[:, :], rl_d[:, :]).then_inc(s_ld[3], 16)
            for i in range(2, nslab):
                xt_load(i)
                if i >= 2:
                    h_store(i - 2)
            h_store(nslab - 2)
            h_store(nslab - 1)
            # bias row (after ALL h writes are complete)
            for k in range(2):
                sync.wait_ge(s_hw[k], hw_total[k])
            sync.dma_start(
                h_d[NPAD - 1:NPAD, :], b_d[0:1, :]
            ).then_inc(s_bw, 16)
            # phase B: output stores
            for s in range(NB):
                sync.wait_ge(s_ocp, s + 1)
                sync.dma_start(o_d[s], o_sb[:, s % 2, :]).then_inc(
                    s_ow[s % 2], 16
                )

        @block.gpsimd
        def _(gpsimd):
            gpsimd.wait_ge(s_ld[2], 16)
            for k in range(2):
                gpsimd.wait_ge(s_hw[k], hw_total[k])
            gpsimd.wait_ge(s_bw, 16)
            ioff = 0
            for s in range(NB):
                n = n128[s]
                if s >= VAL_BUFS:
                    gpsimd.wait_ge(s_pmm, int(cum[s - VAL_BUFS + 1]))
                gpsimd.dma_gather(
                    val_sb[:, s % VAL_BUFS, 0:chunks[s], :],
                    h_d[:, :],
                    col_sb[:, ioff // 16:(ioff + n) // 16],
                    n,
                    n,
                    128,
                    single_packet=False,
                ).then_inc(s_gat[s % VAL_BUFS], 16)
                ioff += n

        @block.tensor
        def _(tensor):
            tensor.wait_ge(s_ld[0], 16)
            # phase A: h tile t = xT_t^T @ W  (two K chunks)
            for t in range(NT):
                i = t // SLAB
                g = t // PSG
                if t % SLAB == 0:
                    tensor.wait_ge(s_xt[i % XT_BUFS], 16 * (i // XT_BUFS + 1))
                if t % PSG == 0 and g >= PH_BANKS:
                    tensor.wait_ge(s_hcp, g - (PH_BANKS - 1))
                pos = t % PSG
                tensor.matmul(
                    ph[g % PH_BANKS][:, pos * 128:(pos + 1) * 128],
                    xt_sb[:, i % XT_BUFS, t - i * SLAB, 0, :],
                    w_sb[:, 0, :],
                    start=True,
                    stop=False,
                )
                tensor.matmul(
                    ph[g % PH_BANKS][:, pos * 128:(pos + 1) * 128],
                    xt_sb[:, i % XT_BUFS, t - i * SLAB, 1, :],
                    w_sb[:, 1, :],
                    start=False,
                    stop=True,
                ).then_inc(s_hmm, 1)
            # phase B: out_slot += S_chunk^T @ val_chunk
            for s in range(NB):
                if s >= 2:
                    tensor.wait_ge(s_ocp, s - 1)
                tensor.wait_ge(s_gat[s % VAL_BUFS], 16 * (s // VAL_BUFS + 1))
                for c in range(chunks[s]):
                    j = int(cum[s]) + c
                    tensor.wait_ge(s_s, j + 1)
                    tensor.matmul(
                        pbk[s % 2][:, 0:128],
                        s_sb[:, j % S_BUFS, :],
                        val_sb[:, s % VAL_BUFS, c, :],
                        start=(c == 0),
                        stop=(c == chunks[s] - 1),
                    ).then_inc(s_pmm, 1)

        @block.vector
        def _(vector):
            # phase A: PSUM fp32 -> SBUF fp16, PSG tiles per op
            for g, (t0, nt) in enumerate(pgroups):
                k = t0 // SLAB
                vector.wait_ge(s_hmm, t0 + nt)
                if k >= 2 and t0 % SLAB == 0:
                    vector.wait_ge(s_hw[k % 2], 16 * (k // 2))
                u = t0 - k * SLAB
                vector.tensor_copy(
                    h16_sb[:, k % 2, u:u + nt, :].rearrange("p t f -> p (t f)"),
                    ph[g % PH_BANKS][:, 0:nt * 128],
                ).then_inc(s_hcp, 1)
            # phase B: one-hot tiles S[e, n] = (iota[n] == rowloc[e])
            vector.wait_ge(s_ld[1], 16)
            vector.wait_ge(s_ld[3], 16)
            for j in range(tot_ch):
                if j >= S_BUFS:
                    vector.wait_ge(s_pmm, j - (S_BUFS - 1))
                vector.tensor_scalar(
                    s_sb[:, j % S_BUFS, :],
                    iota_sb[:, :],
                    rl_sb[:, j:j + 1],
                    None,
                    mybir.AluOpType.is_equal,
                ).then_inc(s_s, 1)

        @block.scalar
        def _(scalar):
            for s in range(NB):
                scalar.wait_ge(s_pmm, int(cum[s + 1]))
                if s >= 2:
                    scalar.wait_ge(s_ow[s % 2], 16 * (s // 2))
                scalar.activation(
                    o_sb[:, s % 2, :],
                    pbk[s % 2][:, 0:128],
                    mybir.ActivationFunctionType.Relu,
                ).then_inc(s_ocp, 1)

    nc.compile()
    return nc


def _run(x, edge_index, weight, bias, trace=False):
    xt_pm, w_pm, bias16, iota16, col16, rl32, plan = _host_prep(
        x, edge_index, weight, bias
    )
    nc = _build_program(plan)
    in_maps = [
        {
            "xt": xt_pm,
            "w": w_pm,
            "bias": bias16,
            "iota": iota16,
            "col": np.ascontiguousarray(col16[c]),
            "rl": np.ascontiguousarray(rl32[c]),
        }
        for c in range(NCORES)
    ]
    res = run_bass_kernel_spmd(nc, in_maps, list(range(NCORES)), trace=trace)
    slot_blocks = plan["slot_blocks"]
    full = np.zeros((NBLK, 128, 128), np.float32)
    for c in range(NCORES):
        o = np.asarray(res.results[c]["out"], np.float32).reshape(NB, 128, 128)
        for s in range(NB):
            b = slot_blocks[c, s]
            if b >= 0:
                full[b] = o[s]
    out = full.reshape(NPAD, 128)[:N_NODES]
    return np.ascontiguousarray(out), res


def kernel(x, edge_index, weight, bias):
    out, _ = _run(x, edge_index, weight, bias, trace=False)
    return out


# revision 33
# speedup vs baseline: 1.9789x; 1.0526x over previous
"""GNN message-passing (graph convolution) kernel for 8 Trainium2 NeuronCores.

    out = relu(segment_sum(h[col], row) + bias),  h = x @ W

Strategy (dst-block sharding -- no collectives):
  * Host sorts edges by destination node into 157 blocks of 128 dst nodes;
    blocks are assigned contiguously to cores (20/core).  Within a core the
    blocks are ordered biggest-first into "slots" so that the per-slot
    num_idxs (max over cores) carries minimal padding; the host un-permutes
    the output rows afterwards.
  * Phase A (replicated): h = x @ W on the PE in fp16 (PSUM fp32).  x ships
    pre-transposed partition-major [128k, NT, 2, 128n] so slab loads are
    16-tile (8KB-elem) DMAs.  h is written to DRAM TILE-MAJOR (row p*NT+t
    holds node t*128+p) so slab stores are 4KB-contiguous per partition --
    twice the DMA rate of row-major 256B stores.  Gather indices are
    permuted on the host to match (idx' = (r%128)*NT + r//128).
  * Phase B: one SWDGE dma_gather per dst-block slot (~4.3K idxs) fetches
    h rows edge-on-partition [128e, chunks, 128f]; the DVE builds one-hot
    tiles S[e,n] = (iota == rowloc) fp16; the PE computes
    out_block += S^T @ val accumulated over chunks in PSUM fp32 (an exact
    segment-sum).  Bias rides as 128 leading idxs per slot that gather a
    bias row stored at the end of h with an identity one-hot.  ACT applies
    ReLU PSUM->SBUF; results DMA out per slot.

Numerics: fp16 operands with fp32 accumulation everywhere; one-hot matmul is
exact, so the only error is fp16 rounding of x, W and h (~3e-4 relative).
"""

import sys

import numpy as np

sys.path.insert(0, "/opt/trn_rl_repo")

import concourse.bacc as bacc  # noqa: E402
import concourse.mybir as mybir  # noqa: E402
from concourse.bass_utils import run_bass_kernel_spmd  # noqa: E402

N_NODES = 20000
FIN = 256
FOUT = 128
N_EDGES = 640000

NT = 157                 # node tiles of 128 (nodes padded to 20096)
NPAD = NT * 128          # 20096
NBLK = 157               # dst blocks of 128 nodes
NCORES = 8
NB = 20                  # block slots per core (core 7: 17 real + 3 dummy)
BIAS_NODE = NPAD - 1     # node whose h row phase-B reads the bias vector from

SLAB = 16                # phase-A tiles per xt load slab
ST_SLAB = 8              # phase-A tiles per h store slab
PSG = 4                  # phase-A tiles per PSUM bank group (cast batch)
XT_BUFS = 4              # xt slab ring (phase A)
PH_BANKS = 4             # phase-A PSUM banks
VAL_BUFS = 3             # gathered-value ring (phase B)
S_BUFS = 8               # one-hot tile ring (phase B)

FP16 = mybir.dt.float16
FP32 = mybir.dt.float32
I16 = mybir.dt.int16


def _idxp(r):
    """Node index -> tile-major h row index: row = (r%128)*NT + r//128."""
    return (r % 128) * NT + r // 128


def _host_prep(x, edge_index, weight, bias):
    """Cast/retile operands and bucket edges by destination block."""
    x = np.asarray(x, np.float32)
    weight = np.asarray(weight, np.float32)
    bias = np.asarray(bias, np.float32)

    xpad = np.zeros((NPAD, FIN), np.float32)
    xpad[:N_NODES] = x
    # lhsT partition-major: xt[k, t, kc, n] = x[t*128+n, kc*128+k]
    xt_pm = np.ascontiguousarray(
        xpad.reshape(NT, 128, 2, 128).transpose(3, 0, 2, 1).astype(np.float16)
    )
    # W partition-major: w[k, kc, f] = weight[kc*128+k, f]
    w_pm = np.ascontiguousarray(
        weight.reshape(2, 128, 128).transpose(1, 0, 2).astype(np.float16)
    )
    bias16 = np.ascontiguousarray(bias.astype(np.float16).reshape(1, 128))
    iota16 = np.ascontiguousarray(
        np.broadcast_to(np.arange(128, dtype=np.float16), (128, 128))
    )

    row = np.asarray(edge_index[0]).astype(np.int64)
    col = np.asarray(edge_index[1]).astype(np.int64)
    order = np.argsort(row, kind="stable")
    rs = row[order].astype(np.int32)
    cs = col[order].astype(np.int32)

    blk = rs >> 7
    counts = np.bincount(blk, minlength=NBLK)
    starts = np.concatenate([[0], np.cumsum(counts)])

    # Deal blocks to cores by global size rank (biggest first, round-robin):
    # every core's slot-s block has nearly the same size, so the per-slot
    # num_idxs (a max over cores) carries minimal padding.
    slot_blocks = np.full((NCORES, NB), -1, np.int64)  # block id per (c, s)
    ranked = sorted(range(NBLK), key=lambda b: -counts[b])
    for r, b in enumerate(ranked):
        slot_blocks[r % NCORES, r // NCORES] = b

    # A bias chunk per slot (128 gathered bias rows + identity one-hot) is
    # only needed when bias is nonzero; the harness's bias is zeros.
    use_bias = bool(np.any(bias != 0.0))
    boff = 128 if use_bias else 0
    bias_rl = np.arange(128, dtype=np.float32)
    pad_idx = _idxp(BIAS_NODE) if use_bias else 0

    # Per-slot edge lists (tile-major gather idx + in-block dst), per core.
    seg_cols = {}  # (c, s) -> int32 idx array (bias-prefixed if use_bias)
    seg_rls = {}
    for c in range(NCORES):
        for s in range(NB):
            b = slot_blocks[c, s]
            if b >= 0:
                e0, e1 = int(starts[b]), int(starts[b + 1])
                cols = cs[e0:e1].astype(np.int64)
                idx = (cols % 128) * NT + cols // 128
                rl = (rs[e0:e1] - b * 128).astype(np.float32)
            else:
                idx = np.zeros(0, np.int64)
                rl = np.zeros(0, np.float32)
            if use_bias:
                idx = np.concatenate([np.full(128, pad_idx, np.int64), idx])
                rl = np.concatenate([bias_rl, rl])
            seg_cols[(c, s)] = idx
            seg_rls[(c, s)] = rl

    # Gather units. Early-split the first slots by source tile so their
    # "early" part (cols in tiles [0, TSPLIT)) can gather while phase A is
    # still storing the last h slabs; chunk-split the final slot so its
    # matmul tail overlaps its own second-half transfer.
    TSPLIT = 144
    early_slots = {0, 1} if not use_bias else set()
    units = []           # emission-ordered: (slot, part) part: 0=early,1=main
    for s in sorted(early_slots):
        units.append((s, 0))
    for s in range(NB):
        if s in early_slots:
            units.append((s, 1))
        elif s == NB - 1:
            units.append((s, 1))
            units.append((s, 2))
        else:
            units.append((s, 1))

    def rup(v):
        return max(128, int(-(-v // 128) * 128))

    # per-(c, s) split edge arrays and per-unit padded sizes
    split_cols = {}
    split_rls = {}
    for s in range(NB):
        if s in early_slots:
            for c in range(NCORES):
                idx = seg_cols[(c, s)]
                m = (idx % NT) < TSPLIT
                split_cols[(c, s, 0)] = idx[m]
                split_rls[(c, s, 0)] = seg_rls[(c, s)][m]
                split_cols[(c, s, 1)] = idx[~m]
                split_rls[(c, s, 1)] = seg_rls[(c, s)][~m]
        else:
            for c in range(NCORES):
                split_cols[(c, s, 1)] = seg_cols[(c, s)]
                split_rls[(c, s, 1)] = seg_rls[(c, s)]

    un = {}              # (s, part) -> padded idx count (mult of 128)
    for s in range(NB):
        if s in early_slots:
            un[(s, 0)] = rup(max(len(split_cols[(c, s, 0)]) for c in range(NCORES)))
            un[(s, 1)] = rup(max(len(split_cols[(c, s, 1)]) for c in range(NCORES)))
        elif s == NB - 1:
            n = rup(max(len(split_cols[(c, s, 1)]) for c in range(NCORES)))
            un[(s, 1)] = max(128, n // 256 * 128)
            un[(s, 2)] = n - un[(s, 1)]
        else:
            un[(s, 1)] = rup(max(len(split_cols[(c, s, 1)]) for c in range(NCORES)))
    # slot 19's chunk-split shares one edge list; re-split it by count
    s19 = NB - 1
    for c in range(NCORES):
        idx = split_cols[(c, s19, 1)]
        rl = split_rls[(c, s19, 1)]
        k = min(len(idx), un[(s19, 1)])
        split_cols[(c, s19, 1)], split_cols[(c, s19, 2)] = idx[:k], idx[k:]
        split_rls[(c, s19, 1)], split_rls[(c, s19, 2)] = rl[:k], rl[k:]

    chunks = [
        sum(un[(s, p)] for (us, p) in units if us == s) // 128
        for s in range(NB)
    ]
    tot_idx = sum(un[u] for u in units)
    tot_ch = sum(chunks)

    col16 = np.zeros((NCORES, 128, tot_idx // 16), np.int16)
    rl16 = np.full((NCORES, 128, tot_ch), -1.0, np.float32)

    # col layout follows unit EMISSION order; rl layout is slot-major chunks
    uinfo = []           # per unit: dict(slot, part, n, ioff, ch0)
    ioff = 0
    slot_ch_used = [0] * NB
    cum0 = np.concatenate([[0], np.cumsum(chunks)])
    for (s, p) in units:
        n = un[(s, p)]
        ch0 = slot_ch_used[s]
        uinfo.append({"slot": s, "part": p, "n": n, "ioff": ioff, "ch0": ch0})
        for c in range(NCORES):
            idx = split_cols[(c, s, p)]
            rl = split_rls[(c, s, p)]
            lin_col = np.full(n, pad_idx, np.int32)
            lin_col[: len(idx)] = idx
            lin_rl = np.full(n, -1.0, np.float32)
            lin_rl[: len(rl)] = rl
            col16[c, :, ioff // 16:(ioff + n) // 16] = np.tile(
                lin_col.reshape(n // 16, 16).T.astype(np.int16), (8, 1)
            )
            choff = int(cum0[s]) + ch0
            rl16[c, :, choff:choff + n // 128] = lin_rl.reshape(n // 128, 128).T
        ioff += n
        slot_ch_used[s] += n // 128

    plan = {
        "chunks": chunks,
        "slot_blocks": slot_blocks,
        "units": uinfo,
        "use_bias": use_bias,
        "tsplit": TSPLIT,
    }
    return xt_pm, w_pm, bias16, iota16, col16, rl16, plan


GAT_LANES = 8


def _build_program(plan):
    chunks = plan["chunks"]
    units = plan["units"]
    use_bias = plan["use_bias"]
    tsplit = plan["tsplit"]
    chmax = max(chunks)
    tot_idx = sum(u["n"] for u in units)
    tot_ch = sum(chunks)
    cum = np.concatenate([[0], np.cumsum(chunks)])  # cum[s] = chunks before s
    # per-unit completion-sem lane and per-lane fire ordinal
    lane_fire = [0] * GAT_LANES
    for ui, u in enumerate(units):
        u["lane"] = ui % GAT_LANES
        lane_fire[u["lane"]] += 1
        u["fire"] = lane_fire[u["lane"]]
    by_slot = {}         # slot -> its units in chunk order
    for u in units:
        by_slot.setdefault(u["slot"], []).append(u)
    for s in by_slot:
        by_slot[s].sort(key=lambda u: u["ch0"])

    nc = bacc.Bacc("TRN2")

    xt_d = nc.dram_tensor("xt", [128, NT, 2, 128], FP16, kind="ExternalInput")
    w_d = nc.dram_tensor("w", [128, 2, 128], FP16, kind="ExternalInput")
    b_d = nc.dram_tensor("bias", [1, 128], FP16, kind="ExternalInput")
    io_d = nc.dram_tensor("iota", [128, 128], FP16, kind="ExternalInput")
    col_d = nc.dram_tensor("col", [128, tot_idx // 16], I16, kind="ExternalInput")
    rl_d = nc.dram_tensor("rl", [128, tot_ch], FP32, kind="ExternalInput")
    h_d = nc.dram_tensor("hbuf", [NPAD, 128], FP16)  # tile-major rows
    o_d = nc.dram_tensor("out", [NB, 128, 128], FP32, kind="ExternalOutput")

    # phase-A DMA slabs of up to SLAB tiles
    slabs = []
    t = 0
    while t < NT:
        nt = min(SLAB, NT - t)
        slabs.append((t, nt))
        t += nt
    nslab = len(slabs)
    # phase-A store slabs of up to ST_SLAB tiles
    st_slabs = []
    t = 0
    while t < NT:
        nt = min(ST_SLAB, NT - t)
        st_slabs.append((t, nt))
        t += nt
    nst = len(st_slabs)
    # phase-A psum groups of up to PSG tiles (aligned inside slabs)
    pgroups = []
    for t0, nt in slabs:
        u = 0
        while u < nt:
            np_ = min(PSG, nt - u)
            pgroups.append((t0 + u, np_))
            u += np_
    npg = len(pgroups)
    # cast count per slab prefix (casts are 1 per pgroup)
    pg_of_tile_end = {}  # tile_end -> number of pgroups fully before it
    acc = 0
    for g0, gn in pgroups:
        acc += 1
        pg_of_tile_end[g0 + gn] = acc

    from contextlib import ExitStack

    with ExitStack() as es:
        ph = [
            es.enter_context(nc.psum_tensor(f"ph{k}", [128, 512], FP32))
            for k in range(PH_BANKS)
        ]
        pbk = [
            es.enter_context(nc.psum_tensor(f"pb{k}", [128, 512], FP32))
            for k in range(2)
        ]
        w_sb = es.enter_context(nc.sbuf_tensor("w_sb", [128, 2, 128], FP16))
        xt_sb = es.enter_context(
            nc.sbuf_tensor("xt_sb", [128, XT_BUFS, SLAB, 2, 128], FP16)
        )
        h16_sb = es.enter_context(
            nc.sbuf_tensor("h16_sb", [128, 4, ST_SLAB, 128], FP16)
        )
        iota_sb = es.enter_context(nc.sbuf_tensor("iota_sb", [128, 128], FP16))
        col_sb = es.enter_context(
            nc.sbuf_tensor("col_sb", [128, tot_idx // 16], I16)
        )
        rl_sb = es.enter_context(nc.sbuf_tensor("rl_sb", [128, tot_ch], FP32))
        val_sb = es.enter_context(
            nc.sbuf_tensor("val_sb", [128, VAL_BUFS, chmax, 128], FP16)
        )
        s_sb = es.enter_context(nc.sbuf_tensor("s_sb", [128, S_BUFS, 128], FP16))
        o_sb = es.enter_context(nc.sbuf_tensor("o_sb", [128, 2, 128], FP32))

        s_ld = [es.enter_context(nc.semaphore(f"s_ld{k}")) for k in range(4)]
        s_xt = [
            es.enter_context(nc.semaphore(f"s_xt{k}")) for k in range(XT_BUFS)
        ]
        s_hw = [es.enter_context(nc.semaphore(f"s_hw{k}")) for k in range(4)]
        s_bw = es.enter_context(nc.semaphore("s_bw"))
        s_gat = [
            es.enter_context(nc.semaphore(f"s_gat{k}")) for k in range(GAT_LANES)
        ]
        s_ow = [es.enter_context(nc.semaphore(f"s_ow{k}")) for k in range(2)]
        s_cmz = es.enter_context(nc.semaphore("s_cmz"))
        s_hmm = es.enter_context(nc.semaphore("s_hmm"))
        s_hcp = es.enter_context(nc.semaphore("s_hcp"))
        s_s = es.enter_context(nc.semaphore("s_s"))
        s_pmm = es.enter_context(nc.semaphore("s_pmm"))
        s_ocp = es.enter_context(nc.semaphore("s_ocp"))
        block = es.enter_context(nc.Block())

        h_tm = h_d[:, :].rearrange("(p t) f -> p t f", p=128)  # tile-major view
        hw_total = [0, 0, 0, 0]
        for k, (t0, nt) in enumerate(st_slabs):
            hw_total[k % 4] += 16

        @block.sync
        def _(sync):
            # one-time loads (xt slab 0 ahead of the big col table so the PE
            # starts early; col/rl only matter at the phase-B boundary)
            sync.dma_start(w_sb[:, :, :], w_d[:, :, :]).then_inc(s_ld[0], 16)
            sync.dma_start(iota_sb[:, :], io_d[:, :]).then_inc(s_ld[1], 16)

            def xt_load(i):
                t0, nt = slabs[i]
                if i >= XT_BUFS:
                    pt0, pnt = slabs[i - XT_BUFS]
                    sync.wait_ge(s_hmm, pt0 + pnt)
                sync.dma_start(
                    xt_sb[:, i % XT_BUFS, 0:nt, :, :], xt_d[:, t0:t0 + nt, :, :]
                ).then_inc(s_xt[i % XT_BUFS], 16)

            def h_store(k):
                kt0, knt = st_slabs[k]
                sync.wait_ge(s_hcp, pg_of_tile_end[kt0 + knt])
                sync.dma_start(
                    h_tm[:, kt0:kt0 + knt, :], h16_sb[:, k % 4, 0:knt, :]
                ).then_inc(s_hw[k % 4], 16)

            xt_load(0)
            xt_load(1)
            # only the first two 16-partition groups are read by the SWDGE
            # tx/rx descriptor generators
            sync.dma_start(col_sb[0:32, :], col_d[0:32, :]).then_inc(
                s_ld[2], 16
            )
            sync.dma_start(rl_sb[:, :], rl_d[:, :]).then_inc(s_ld[3], 16)
            st_ptr = 0
            for i in range(2, nslab):
                xt_load(i)
                # store slabs fully covered by loaded-and-computed tiles
                lim = slabs[i - 2][0] + slabs[i - 2][1]
                while st_ptr < nst and (
                    st_slabs[st_ptr][0] + st_slabs[st_ptr][1] <= lim
                ):
                    h_store(st_ptr)
                    st_ptr += 1
            while st_ptr < nst:
                h_store(st_ptr)
                st_ptr += 1
            # bias row (after ALL h writes are complete)
            for k in range(4):
                sync.wait_ge(s_hw[k], hw_total[k])
            sync.dma_start(
                h_d[NPAD - 1:NPAD, :], b_d[0:1, :]
            ).then_inc(s_bw, 16)
            # phase B: output stores
            for s in range(NB):
                sync.wait_ge(s_ocp, s + 1)
                sync.dma_start(o_d[s], o_sb[:, s % 2, :]).then_inc(
                    s_ow[s % 2], 16
                )

        @block.gpsimd
        def _(gpsimd):
            # stores covering source tiles [0, tsplit) -- what "early" gather
            # units depend on
            hw_part = [0, 0, 0, 0]
            for k, (t0, nt) in enumerate(st_slabs):
                if t0 + nt <= tsplit:
                    hw_part[k % 4] += 16
            gpsimd.wait_ge(s_ld[2], 16)
            gpsimd.wait_ge(s_cmz, 1)
            full_waited = False
            for u in units:
                s = u["slot"]
                n = u["n"]
                if u["part"] == 0:
                    for k in range(4):
                        gpsimd.wait_ge(s_hw[k], hw_part[k])
                elif not full_waited:
                    for k in range(4):
                        gpsimd.wait_ge(s_hw[k], hw_total[k])
                    gpsimd.wait_ge(s_bw, 16)
                    full_waited = True
                if s >= VAL_BUFS:
                    gpsimd.wait_ge(s_pmm, int(cum[s - VAL_BUFS + 1]))
                ch0 = u["ch0"]
                gpsimd.dma_gather(
                    val_sb[:, s % VAL_BUFS, ch0:ch0 + n // 128, :],
                    h_d[:, :],
                    col_sb[:, u["ioff"] // 16:(u["ioff"] + n) // 16],
                    n,
                    n,
                    128,
                    single_packet=False,
                ).then_inc(s_gat[u["lane"]], 16)

        @block.tensor
        def _(tensor):
            tensor.wait_ge(s_ld[0], 16)
            # phase A: h tile t = xT_t^T @ W  (two K chunks)
            for t in range(NT):
                i = t // SLAB
                g = t // PSG
                if t % SLAB == 0:
                    tensor.wait_ge(s_xt[i % XT_BUFS], 16 * (i // XT_BUFS + 1))
                if t % PSG == 0 and g >= PH_BANKS:
                    tensor.wait_ge(s_hcp, g - (PH_BANKS - 1))
                pos = t % PSG
                tensor.matmul(
                    ph[g % PH_BANKS][:, pos * 128:(pos + 1) * 128],
                    xt_sb[:, i % XT_BUFS, t - i * SLAB, 0, :],
                    w_sb[:, 0, :],
                    start=True,
                    stop=False,
                )
                tensor.matmul(
                    ph[g % PH_BANKS][:, pos * 128:(pos + 1) * 128],
                    xt_sb[:, i % XT_BUFS, t - i * SLAB, 1, :],
                    w_sb[:, 1, :],
                    start=False,
                    stop=True,
                ).then_inc(s_hmm, 1)
            # phase B: out_slot += S_chunk^T @ val_chunk
            for s in range(NB):
                if s >= 2:
                    tensor.wait_ge(s_ocp, s - 1)
                ubound = {
                    u["ch0"]: u for u in by_slot[s]
                }
                for c in range(chunks[s]):
                    if c in ubound:
                        u = ubound[c]
                        tensor.wait_ge(s_gat[u["lane"]], 16 * u["fire"])
                    j = int(cum[s]) + c
                    tensor.wait_ge(s_s, j + 1)
                    tensor.matmul(
                        pbk[s % 2][:, 0:128],
                        s_sb[:, j % S_BUFS, :],
                        val_sb[:, s % VAL_BUFS, c, :],
                        start=(c == 0),
                        stop=(c == chunks[s] - 1),
                    ).then_inc(s_pmm, 1)

        @block.vector
        def _(vector):
            # partitions >= 32 of the idx table are never DMA-loaded (SWDGE
            # only reads the first two 16-partition groups); zero them so
            # they hold valid row indices
            vector.memset(col_sb[32:64, :], 0)
            vector.memset(col_sb[64:128, :], 0).then_inc(s_cmz, 1)
            # phase A: PSUM fp32 -> SBUF fp16, PSG tiles per op
            for g, (t0, nt) in enumerate(pgroups):
                k = t0 // ST_SLAB
                vector.wait_ge(s_hmm, t0 + nt)
                if k >= 4 and t0 % ST_SLAB == 0:
                    vector.wait_ge(s_hw[k % 4], 16 * (k // 4))
                u = t0 - k * ST_SLAB
                vector.tensor_copy(
                    h16_sb[:, k % 4, u:u + nt, :].rearrange("p t f -> p (t f)"),
                    ph[g % PH_BANKS][:, 0:nt * 128],
                ).then_inc(s_hcp, 1)
            # phase B: one-hot tiles S[e, n] = (iota[n] == rowloc[e])
            vector.wait_ge(s_ld[1], 16)
            vector.wait_ge(s_ld[3], 16)
            for j in range(tot_ch):
                if j >= S_BUFS:
                    vector.wait_ge(s_pmm, j - (S_BUFS - 1))
                vector.tensor_scalar(
                    s_sb[:, j % S_BUFS, :],
                    iota_sb[:, :],
                    rl_sb[:, j:j + 1],
                    None,
                    mybir.AluOpType.is_equal,
                ).then_inc(s_s, 1)

        @block.scalar
        def _(scalar):
            for s in range(NB):
                scalar.wait_ge(s_pmm, int(cum[s + 1]))
                if s >= 2:
                    scalar.wait_ge(s_ow[s % 2], 16 * (s // 2))
                scalar.activation(
                    o_sb[:, s % 2, :],
                    pbk[s % 2][:, 0:128],
                    mybir.ActivationFunctionType.Relu,
                ).then_inc(s_ocp, 1)

    nc.compile()
    return nc


def _run(x, edge_index, weight, bias, trace=False):
    xt_pm, w_pm, bias16, iota16, col16, rl16, plan = _host_prep(
        x, edge_index, weight, bias
    )
    nc = _build_program(plan)
    in_maps = [
        {
            "xt": xt_pm,
            "w": w_pm,
            "bias": bias16,
            "iota": iota16,
            "col": np.ascontiguousarray(col16[c]),
            "rl": np.ascontiguousarray(rl16[c]),
        }
        for c in range(NCORES)
    ]
    res = run_bass_kernel_spmd(nc, in_maps, list(range(NCORES)), trace=trace)
    slot_blocks = plan["slot_blocks"]
    full = np.zeros((NBLK, 128, 128), np.float32)
    for c in range(NCORES):
        o = np.asarray(res.results[c]["out"], np.float32).reshape(NB, 128, 128)
        for s in range(NB):
            b = slot_blocks[c, s]
            if b >= 0:
                full[b] = o[s]
    out = full.reshape(NPAD, 128)[:N_NODES]
    return np.ascontiguousarray(out), res


def kernel(x, edge_index, weight, bias):
    out, _ = _run(x, edge_index, weight, bias, trace=False)
    return out


# revision 51
# speedup vs baseline: 2.0311x; 1.0264x over previous
"""GNN message-passing (graph convolution) kernel for 8 Trainium2 NeuronCores.

    out = relu(segment_sum(h[col], row) + bias),  h = x @ W

Strategy (dst-block sharding -- no collectives):
  * Host sorts edges by destination node into 157 blocks of 128 dst nodes;
    blocks are assigned contiguously to cores (20/core).  Within a core the
    blocks are ordered biggest-first into "slots" so that the per-slot
    num_idxs (max over cores) carries minimal padding; the host un-permutes
    the output rows afterwards.
  * Phase A (replicated): h = x @ W on the PE in fp16 (PSUM fp32).  x ships
    pre-transposed partition-major [128k, NT, 2, 128n] so slab loads are
    16-tile (8KB-elem) DMAs.  h is written to DRAM TILE-MAJOR (row p*NT+t
    holds node t*128+p) so slab stores are 4KB-contiguous per partition --
    twice the DMA rate of row-major 256B stores.  Gather indices are
    permuted on the host to match (idx' = (r%128)*NT + r//128).
  * Phase B: one SWDGE dma_gather per dst-block slot (~4.3K idxs) fetches
    h rows edge-on-partition [128e, chunks, 128f]; the DVE builds one-hot
    tiles S[e,n] = (iota == rowloc) fp16; the PE computes
    out_block += S^T @ val accumulated over chunks in PSUM fp32 (an exact
    segment-sum).  Bias rides as 128 leading idxs per slot that gather a
    bias row stored at the end of h with an identity one-hot.  ACT applies
    ReLU PSUM->SBUF; results DMA out per slot.

Numerics: fp16 operands with fp32 accumulation everywhere; one-hot matmul is
exact, so the only error is fp16 rounding of x, W and h (~3e-4 relative).
"""

import sys

import numpy as np

sys.path.insert(0, "/opt/trn_rl_repo")

import concourse.bacc as bacc  # noqa: E402
import concourse.mybir as mybir  # noqa: E402
from concourse.bass_utils import run_bass_kernel_spmd  # noqa: E402

N_NODES = 20000
FIN = 256
FOUT = 128
N_EDGES = 640000

NT = 157                 # node tiles of 128 (nodes padded to 20096)
NPAD = NT * 128          # 20096
NBLK = 157               # dst blocks of 128 nodes
NCORES = 8
NB = 20                  # block slots per core (core 7: 17 real + 3 dummy)
BIAS_NODE = NPAD - 1     # node whose h row phase-B reads the bias vector from

SLAB = 16                # phase-A tiles per xt load slab
ST_SLAB = 8              # phase-A tiles per h store slab
PSG = 4                  # phase-A tiles per PSUM bank group (cast batch)
XT_BUFS = 6              # xt slab ring (phase A)
PH_BANKS = 4             # phase-A PSUM banks
VAL_BUFS = 3             # gathered-value ring (phase B)
S_BUFS = 8               # one-hot tile ring (phase B)

FP16 = mybir.dt.float16
FP32 = mybir.dt.float32
I16 = mybir.dt.int16


def _idxp(r):
    """Node index -> tile-major h row index: row = (r%128)*NT + r//128."""
    return (r % 128) * NT + r // 128


def _host_prep(x, edge_index, weight, bias):
    """Cast/retile operands and bucket edges by destination block."""
    x = np.asarray(x, np.float32)
    weight = np.asarray(weight, np.float32)
    bias = np.asarray(bias, np.float32)

    xpad = np.zeros((NPAD, FIN), np.float32)
    xpad[:N_NODES] = x
    # lhsT partition-major: xt[k, t, kc, n] = x[t*128+n, kc*128+k]
    xt_pm = np.ascontiguousarray(
        xpad.reshape(NT, 128, 2, 128).transpose(3, 0, 2, 1).astype(np.float16)
    )
    # W partition-major: w[k, kc, f] = weight[kc*128+k, f]
    w_pm = np.ascontiguousarray(
        weight.reshape(2, 128, 128).transpose(1, 0, 2).astype(np.float16)
    )
    bias16 = np.ascontiguousarray(bias.astype(np.float16).reshape(1, 128))
    iota16 = np.ascontiguousarray(
        np.broadcast_to(np.arange(128, dtype=np.float16), (128, 128))
    )

    row = np.asarray(edge_index[0]).astype(np.int64)
    col = np.asarray(edge_index[1]).astype(np.int64)
    order = np.argsort(row, kind="stable")
    rs = row[order].astype(np.int32)
    cs = col[order].astype(np.int32)

    blk = rs >> 7
    counts = np.bincount(blk, minlength=NBLK)
    starts = np.concatenate([[0], np.cumsum(counts)])

    # Deal blocks to cores by global size rank (biggest first, round-robin):
    # every core's slot-s block has nearly the same size, so the per-slot
    # num_idxs (a max over cores) carries minimal padding.
    slot_blocks = np.full((NCORES, NB), -1, np.int64)  # block id per (c, s)
    ranked = sorted(range(NBLK), key=lambda b: -counts[b])
    for r, b in enumerate(ranked):
        slot_blocks[r % NCORES, r // NCORES] = b

    # A bias chunk per slot (128 gathered bias rows + identity one-hot) is
    # only needed when bias is nonzero; the harness's bias is zeros.
    use_bias = bool(np.any(bias != 0.0))
    boff = 128 if use_bias else 0
    bias_rl = np.arange(128, dtype=np.float32)
    pad_idx = _idxp(BIAS_NODE) if use_bias else 0

    # Per-slot edge lists (tile-major gather idx + in-block dst), per core.
    seg_cols = {}  # (c, s) -> int32 idx array (bias-prefixed if use_bias)
    seg_rls = {}
    for c in range(NCORES):
        for s in range(NB):
            b = slot_blocks[c, s]
            if b >= 0:
                e0, e1 = int(starts[b]), int(starts[b + 1])
                cols = cs[e0:e1].astype(np.int64)
                idx = (cols % 128) * NT + cols // 128
                rl = (rs[e0:e1] - b * 128).astype(np.float32)
            else:
                idx = np.zeros(0, np.int64)
                rl = np.zeros(0, np.float32)
            if use_bias:
                idx = np.concatenate([np.full(128, pad_idx, np.int64), idx])
                rl = np.concatenate([bias_rl, rl])
            seg_cols[(c, s)] = idx
            seg_rls[(c, s)] = rl

    # Gather units. Early-split the first slots by source tile so "early"
    # parts (cols in tiles [0, tmax)) gather while phase A is still storing
    # later h slabs; chunk-split the final slot so its matmul tail overlaps
    # its own later transfers.
    tiers = {0: [128], 1: [128]} if not use_bias else {}

    def rup(v):
        return max(128, int(-(-v // 128) * 128))

    # per-(c, slot, part) edge arrays; part p covers tiles
    # [bounds[p], bounds[p+1])
    split_cols = {}
    split_rls = {}
    nparts = {}
    tmax = {}            # (s, part) -> tile prefix this unit depends on
    for s in range(NB):
        ts = tiers.get(s, [])
        bounds = [0] + ts + [NT]
        nparts[s] = len(bounds) - 1
        for p in range(nparts[s]):
            tmax[(s, p)] = bounds[p + 1] if p < len(ts) else NT
        for c in range(NCORES):
            idx = seg_cols[(c, s)]
            rl = seg_rls[(c, s)]
            tile = idx % NT
            for p in range(nparts[s]):
                m = (tile >= bounds[p]) & (tile < bounds[p + 1])
                split_cols[(c, s, p)] = idx[m]
                split_rls[(c, s, p)] = rl[m]

    un = {}              # (s, part) -> padded idx count (mult of 128)
    for s in range(NB):
        for p in range(nparts[s]):
            un[(s, p)] = rup(
                max(len(split_cols[(c, s, p)]) for c in range(NCORES))
            )
    # chunk-split the final slot's last part into thirds (tail overlap)
    s19 = NB - 1
    lp = nparts[s19] - 1
    n = un[(s19, lp)]
    third = max(128, n // 384 * 128)
    if n >= 3 * 128:
        sizes19 = [n - third - 512, third, 512] if n >= third + 640 else [
            n - 2 * third, third, third
        ]
        for c in range(NCORES):
            idx = split_cols[(c, s19, lp)]
            rl = split_rls[(c, s19, lp)]
            k1 = min(len(idx), sizes19[0])
            k2 = min(len(idx), sizes19[0] + sizes19[1])
            for q, (a, b) in enumerate([(0, k1), (k1, k2), (k2, len(idx))]):
                split_cols[(c, s19, lp + q)] = idx[a:b]
                split_rls[(c, s19, lp + q)] = rl[a:b]
        for q in range(3):
            un[(s19, lp + q)] = sizes19[q]
            tmax[(s19, lp + q)] = NT
        nparts[s19] = lp + 3

    # emission order: earliest-firing units first, then by slot
    units = sorted(
        [(s, p) for s in range(NB) for p in range(nparts[s])],
        key=lambda sp: (tmax[sp], sp[0], sp[1]),
    )

    chunks = [
        sum(un[(s, p)] for (us, p) in units if us == s) // 128
        for s in range(NB)
    ]
    tot_idx = sum(un[u] for u in units)
    tot_ch = sum(chunks)

    col16 = np.zeros((NCORES, 128, tot_idx // 16), np.int16)
    rl16 = np.full((NCORES, 128, tot_ch), -1.0, np.float32)

    # col layout follows unit EMISSION order; rl layout is slot-major chunks
    uinfo = []           # per unit: dict(slot, part, n, ioff, ch0)
    ioff = 0
    slot_ch_used = [0] * NB
    cum0 = np.concatenate([[0], np.cumsum(chunks)])
    for (s, p) in units:
        n = un[(s, p)]
        ch0 = slot_ch_used[s]
        uinfo.append(
            {
                "slot": s,
                "part": p,
                "n": n,
                "ioff": ioff,
                "ch0": ch0,
                "tmax": tmax[(s, p)],
            }
        )
        for c in range(NCORES):
            idx = split_cols[(c, s, p)]
            rl = split_rls[(c, s, p)]
            lin_col = np.full(n, pad_idx, np.int32)
            lin_col[: len(idx)] = idx
            lin_rl = np.full(n, -1.0, np.float32)
            lin_rl[: len(rl)] = rl
            col16[c, :, ioff // 16:(ioff + n) // 16] = np.tile(
                lin_col.reshape(n // 16, 16).T.astype(np.int16), (8, 1)
            )
            choff = int(cum0[s]) + ch0
            rl16[c, :, choff:choff + n // 128] = lin_rl.reshape(n // 128, 128).T
        ioff += n
        slot_ch_used[s] += n // 128

    plan = {
        "chunks": chunks,
        "slot_blocks": slot_blocks,
        "units": uinfo,
        "use_bias": use_bias,
    }
    return xt_pm, w_pm, bias16, iota16, col16, rl16, plan


GAT_LANES = 8


def _build_program(plan):
    chunks = plan["chunks"]
    units = plan["units"]
    use_bias = plan["use_bias"]
    chmax = max(chunks)
    tot_idx = sum(u["n"] for u in units)
    tot_ch = sum(chunks)
    cum = np.concatenate([[0], np.cumsum(chunks)])  # cum[s] = chunks before s
    # per-unit completion-sem lane and per-lane fire ordinal
    lane_fire = [0] * GAT_LANES
    for ui, u in enumerate(units):
        u["lane"] = ui % GAT_LANES
        lane_fire[u["lane"]] += 1
        u["fire"] = lane_fire[u["lane"]]
    by_slot = {}         # slot -> its units in chunk order
    for u in units:
        by_slot.setdefault(u["slot"], []).append(u)
    for s in by_slot:
        by_slot[s].sort(key=lambda u: u["ch0"])

    nc = bacc.Bacc("TRN2")

    xt_d = nc.dram_tensor("xt", [128, NT, 2, 128], FP16, kind="ExternalInput")
    w_d = nc.dram_tensor("w", [128, 2, 128], FP16, kind="ExternalInput")
    b_d = nc.dram_tensor("bias", [1, 128], FP16, kind="ExternalInput")
    io_d = nc.dram_tensor("iota", [128, 128], FP16, kind="ExternalInput")
    col_d = nc.dram_tensor("col", [128, tot_idx // 16], I16, kind="ExternalInput")
    rl_d = nc.dram_tensor("rl", [128, tot_ch], FP32, kind="ExternalInput")
    h_d = nc.dram_tensor("hbuf", [NPAD, 128], FP16)  # tile-major rows
    o_d = nc.dram_tensor("out", [NB, 128, 128], FP32, kind="ExternalOutput")

    # phase-A DMA slabs of up to SLAB tiles
    slabs = []
    t = 0
    while t < NT:
        nt = min(SLAB, NT - t)
        slabs.append((t, nt))
        t += nt
    nslab = len(slabs)
    # phase-A store slabs of up to ST_SLAB tiles
    st_slabs = []
    t = 0
    while t < NT:
        nt = min(ST_SLAB, NT - t)
        st_slabs.append((t, nt))
        t += nt
    nst = len(st_slabs)
    # phase-A psum groups of up to PSG tiles (aligned inside slabs)
    pgroups = []
    for t0, nt in slabs:
        u = 0
        while u < nt:
            np_ = min(PSG, nt - u)
            pgroups.append((t0 + u, np_))
            u += np_
    npg = len(pgroups)
    # cast count per slab prefix (casts are 1 per pgroup)
    pg_of_tile_end = {}  # tile_end -> number of pgroups fully before it
    acc = 0
    for g0, gn in pgroups:
        acc += 1
        pg_of_tile_end[g0 + gn] = acc

    from contextlib import ExitStack

    with ExitStack() as es:
        ph = [
            es.enter_context(nc.psum_tensor(f"ph{k}", [128, 512], FP32))
            for k in range(PH_BANKS)
        ]
        pbk = [
            es.enter_context(nc.psum_tensor(f"pb{k}", [128, 512], FP32))
            for k in range(2)
        ]
        w_sb = es.enter_context(nc.sbuf_tensor("w_sb", [128, 2, 128], FP16))
        xt_sb = es.enter_context(
            nc.sbuf_tensor("xt_sb", [128, XT_BUFS, SLAB, 2, 128], FP16)
        )
        h16_sb = es.enter_context(
            nc.sbuf_tensor("h16_sb", [128, 20, ST_SLAB, 128], FP16)
        )
        iota_sb = es.enter_context(nc.sbuf_tensor("iota_sb", [128, 128], FP16))
        col_sb = es.enter_context(
            nc.sbuf_tensor("col_sb", [128, tot_idx // 16], I16)
        )
        rl_sb = es.enter_context(nc.sbuf_tensor("rl_sb", [128, tot_ch], FP32))
        val_sb = es.enter_context(
            nc.sbuf_tensor("val_sb", [128, VAL_BUFS, chmax, 128], FP16)
        )
        s_sb = es.enter_context(nc.sbuf_tensor("s_sb", [128, S_BUFS, 128], FP16))
        o_sb = es.enter_context(nc.sbuf_tensor("o_sb", [128, 2, 128], FP32))

        s_ld = [es.enter_context(nc.semaphore(f"s_ld{k}")) for k in range(4)]
        s_xt = [
            es.enter_context(nc.semaphore(f"s_xt{k}")) for k in range(XT_BUFS)
        ]
        s_hw = [es.enter_context(nc.semaphore(f"s_hw{k}")) for k in range(4)]
        s_bw = es.enter_context(nc.semaphore("s_bw"))
        s_gat = [
            es.enter_context(nc.semaphore(f"s_gat{k}")) for k in range(GAT_LANES)
        ]
        s_ow = [es.enter_context(nc.semaphore(f"s_ow{k}")) for k in range(2)]
        s_cmz = es.enter_context(nc.semaphore("s_cmz"))
        s_prep = es.enter_context(nc.semaphore("s_prep"))
        s_hmm = es.enter_context(nc.semaphore("s_hmm"))
        s_hcp = es.enter_context(nc.semaphore("s_hcp"))
        s_s = es.enter_context(nc.semaphore("s_s"))
        s_pmm = es.enter_context(nc.semaphore("s_pmm"))
        s_ocp = es.enter_context(nc.semaphore("s_ocp"))
        block = es.enter_context(nc.Block())

        h_tm = h_d[:, :].rearrange("(p t) f -> p t f", p=128)  # tile-major view
        hw_total = [0, 0, 0, 0]
        for k, (t0, nt) in enumerate(st_slabs):
            hw_total[k % 4] += 16

        @block.sync
        def _(sync):
            # one-time loads (xt slabs ahead of the small/late tables)

            def xt_load(i):
                t0, nt = slabs[i]
                if i >= XT_BUFS:
                    pt0, pnt = slabs[i - XT_BUFS]
                    sync.wait_ge(s_hmm, pt0 + pnt)
                sync.dma_start(
                    xt_sb[:, i % XT_BUFS, 0:nt, :, :], xt_d[:, t0:t0 + nt, :, :]
                ).then_inc(s_xt[i % XT_BUFS], 16)

            def h_store(k):
                kt0, knt = st_slabs[k]
                sync.wait_ge(s_hcp, pg_of_tile_end[kt0 + knt])
                sync.dma_start(
                    h_tm[:, kt0:kt0 + knt, :], h16_sb[:, k, 0:knt, :]
                ).then_inc(s_hw[k % 4], 16)

            xt_load(0)
            sync.dma_start(w_sb[:, :, :], w_d[:, :, :]).then_inc(s_ld[0], 16)
            sync.dma_start(iota_sb[:, :], io_d[:, :]).then_inc(s_ld[1], 16)
            xt_load(1)
            # only the first two 16-partition groups are read by the SWDGE
            # tx/rx descriptor generators
            sync.dma_start(col_sb[0:32, :], col_d[0:32, :]).then_inc(
                s_ld[2], 16
            )
            sync.dma_start(rl_sb[:, :], rl_d[:, :]).then_inc(s_ld[3], 16)
            # all xt loads first (they gate the PE); h slabs buffer in SBUF
            # and drain afterwards at full DMA rate
            for i in range(2, nslab):
                xt_load(i)
            for k in range(nst):
                h_store(k)
            # bias row (after ALL h writes are complete)
            for k in range(4):
                sync.wait_ge(s_hw[k], hw_total[k])
            sync.dma_start(
                h_d[NPAD - 1:NPAD, :], b_d[0:1, :]
            ).then_inc(s_bw, 16)
            # phase B: output stores
            for s in range(NB):
                sync.wait_ge(s_ocp, s + 1)
                sync.dma_start(o_d[s], o_sb[:, s % 2, :]).then_inc(
                    s_ow[s % 2], 16
                )

        @block.gpsimd
        def _(gpsimd):
            def hw_need(tiles):
                # per-lane store-sem counts covering source tiles [0, tiles)
                need = [0, 0, 0, 0]
                for k, (t0, nt) in enumerate(st_slabs):
                    if t0 + nt <= tiles:
                        need[k % 4] += 16
                return need

            gpsimd.wait_ge(s_ld[2], 16)
            gpsimd.wait_ge(s_cmz, 1)

            def gather_args(u):
                n = u["n"]
                ch0 = u["ch0"]
                return (
                    val_sb[:, u["slot"] % VAL_BUFS, ch0:ch0 + n // 128, :],
                    h_d[:, :],
                    col_sb[:, u["ioff"] // 16:(u["ioff"] + n) // 16],
                    n,
                    n,
                    128,
                )

            full_waited = False
            for u in units:
                s = u["slot"]
                if u["tmax"] < NT:
                    for k, v in enumerate(hw_need(u["tmax"])):
                        gpsimd.wait_ge(s_hw[k], v)
                elif not full_waited:
                    for k in range(4):
                        gpsimd.wait_ge(s_hw[k], hw_total[k])
                    gpsimd.wait_ge(s_bw, 16)
                    full_waited = True
                if s >= VAL_BUFS:
                    gpsimd.wait_ge(s_pmm, int(cum[s - VAL_BUFS + 1]))
                gpsimd.dma_gather(
                    *gather_args(u),
                    single_packet=False,
                ).then_inc(s_gat[u["lane"]], 16)

        @block.tensor
        def _(tensor):
            tensor.wait_ge(s_ld[0], 16)
            # phase A: h tile t = xT_t^T @ W  (two K chunks)
            for t in range(NT):
                i = t // SLAB
                g = t // PSG
                if t % SLAB == 0:
                    tensor.wait_ge(s_xt[i % XT_BUFS], 16 * (i // XT_BUFS + 1))
                if t % PSG == 0 and g >= PH_BANKS:
                    tensor.wait_ge(s_hcp, g - (PH_BANKS - 1))
                pos = t % PSG
                tensor.matmul(
                    ph[g % PH_BANKS][:, pos * 128:(pos + 1) * 128],
                    xt_sb[:, i % XT_BUFS, t - i * SLAB, 0, :],
                    w_sb[:, 0, :],
                    start=True,
                    stop=False,
                )
                tensor.matmul(
                    ph[g % PH_BANKS][:, pos * 128:(pos + 1) * 128],
                    xt_sb[:, i % XT_BUFS, t - i * SLAB, 1, :],
                    w_sb[:, 1, :],
                    start=False,
                    stop=True,
                ).then_inc(s_hmm, 1)
            # phase B: out_slot += S_chunk^T @ val_chunk
            for s in range(NB):
                if s >= 2:
                    tensor.wait_ge(s_ocp, s - 1)
                ubound = {
                    u["ch0"]: u for u in by_slot[s]
                }
                for c in range(chunks[s]):
                    if c in ubound:
                        u = ubound[c]
                        tensor.wait_ge(s_gat[u["lane"]], 16 * u["fire"])
                    j = int(cum[s]) + c
                    tensor.wait_ge(s_s, j + 1)
                    tensor.matmul(
                        pbk[s % 2][:, 0:128],
                        s_sb[:, j % S_BUFS, :],
                        val_sb[:, s % VAL_BUFS, c, :],
                        start=(c == 0),
                        stop=(c == chunks[s] - 1),
                    ).then_inc(s_pmm, 1)

        @block.vector
        def _(vector):
            # partitions >= 32 of the idx table are never DMA-loaded (SWDGE
            # only reads the first two 16-partition groups); zero them so
            # they hold valid row indices
            vector.memset(col_sb[32:64, :], 0)
            vector.memset(col_sb[64:128, :], 0).then_inc(s_cmz, 1)
            # phase A: PSUM fp32 -> SBUF fp16, PSG tiles per op
            for g, (t0, nt) in enumerate(pgroups):
                k = t0 // ST_SLAB
                vector.wait_ge(s_hmm, t0 + nt)
                u = t0 - k * ST_SLAB
                vector.tensor_copy(
                    h16_sb[:, k, u:u + nt, :].rearrange("p t f -> p (t f)"),
                    ph[g % PH_BANKS][:, 0:nt * 128],
                ).then_inc(s_hcp, 1)
            # phase B: one-hot tiles S[e, n] = (iota[n] == rowloc[e])
            vector.wait_ge(s_ld[1], 16)
            vector.wait_ge(s_ld[3], 16)
            for j in range(tot_ch):
                if j >= S_BUFS:
                    vector.wait_ge(s_pmm, j - (S_BUFS - 1))
                vector.tensor_scalar(
                    s_sb[:, j % S_BUFS, :],
                    iota_sb[:, :],
                    rl_sb[:, j:j + 1],
                    None,
                    mybir.AluOpType.is_equal,
                ).then_inc(s_s, 1)

        @block.scalar
        def _(scalar):
            for s in range(NB):
                scalar.wait_ge(s_pmm, int(cum[s + 1]))
                if s >= 2:
                    scalar.wait_ge(s_ow[s % 2], 16 * (s // 2))
                scalar.activation(
                    o_sb[:, s % 2, :],
                    pbk[s % 2][:, 0:128],
                    mybir.ActivationFunctionType.Relu,
                ).then_inc(s_ocp, 1)

    nc.compile()
    return nc


def _run(x, edge_index, weight, bias, trace=False):
    xt_pm, w_pm, bias16, iota16, col16, rl16, plan = _host_prep(
        x, edge_index, weight, bias
    )
    nc = _build_program(plan)
    in_maps = [
        {
            "xt": xt_pm,
            "w": w_pm,
            "bias": bias16,
            "iota": iota16,
            "col": np.ascontiguousarray(col16[c]),
            "rl": np.ascontiguousarray(rl16[c]),
        }
        for c in range(NCORES)
    ]
    res = run_bass_kernel_spmd(nc, in_maps, list(range(NCORES)), trace=trace)
    slot_blocks = plan["slot_blocks"]
    full = np.zeros((NBLK, 128, 128), np.float32)
    for c in range(NCORES):
        o = np.asarray(res.results[c]["out"], np.float32).reshape(NB, 128, 128)
        for s in range(NB):
            b = slot_blocks[c, s]
            if b >= 0:
                full[b] = o[s]
    out = full.reshape(NPAD, 128)[:N_NODES]
    return np.ascontiguousarray(out), res


def kernel(x, edge_index, weight, bias):
    out, _ = _run(x, edge_index, weight, bias, trace=False)
    return out
